# revision 47
# baseline (speedup 1.0000x reference)
"""Cross-GNN (3-layer GCN with cross-branch similarity mixing) on 8 trn2 cores.

Sharding: dst nodes across 8 cores. The GCN aggregation
  out = D^-1/2 (A+I) D^-1/2 (X W)
is computed as dense per-src-tile tensor-engine matmuls:
  out^T[f, d] = dinv_d * sum_t hs_t^T @ Acnt[t][:, d]   (PSUM accumulation)
with hs = dinv * (X W) stationary in fp16 and Acnt fp8 edge-count blocks
(self-loops folded into the diagonal) streamed from DRAM contiguously.
The conv matmuls use 128x32 column tiling (4 concurrent PE tiles, one per
t mod 4) accumulating into disjoint 32-partition ranges of one PSUM bank;
the four partials are summed on DVE at eviction. Weight matrices are
replicated; layer boundaries AllGather 32-dim features per branch.
"""
import numpy as np
import ml_dtypes

import concourse.bacc as bacc
import concourse.tile as tile
import concourse.bass as bass
from concourse import mybir
from concourse import bass_utils

N = 50000
F_IN = 256
HID = 32
OUT = 128
EPS = 1e-12
N_CORES = 8
SHARD = N // N_CORES          # 6250

CW = 512                      # column chunk width
NP = 50176                    # padded node count (98 chunks)
NCH = NP // CW                # 98
T = NP // 128                 # 392 src tiles
TG = 28                       # src tiles per A-DMA group
NTG = T // TG                 # 14
CWT = 128                     # tail dst chunk width
DP = 12 * CW + CWT            # 6272 padded per-core dst cols
DCH = 13
CWS = [CW] * 12 + [CWT]       # per-chunk widths
CS = [i * CW for i in range(13)]  # per-chunk col starts
NB = 2                        # node chunks per batched table read

F16 = mybir.dt.float16
F32 = mybir.dt.float32
F8 = mybir.dt.float8e4

_cache = {}


def _segments(col0, ncols):
    """Global col range [col0, col0+ncols) -> list of (core, src_off, dst_off, len)
    where src_off indexes within that core's SHARD-wide slab."""
    segs = []
    pos = col0
    while pos < col0 + ncols:
        c = pos // SHARD
        seg_end = min((c + 1) * SHARD, col0 + ncols)
        segs.append((c, pos - c * SHARD, pos - col0, seg_end - pos))
        pos = seg_end
    return segs


def build_program(variant="full"):
    key = ("nc", variant)
    if key in _cache:
        return _cache[key]
    import contextlib
    from concourse.masks import make_identity

    nc = bacc.Bacc("TRN2", target_bir_lowering=False, debug=False,
                   num_devices=N_CORES, detect_race_conditions=False)

    xT = nc.dram_tensor("xT", [F_IN, NP], F16, kind="ExternalInput").ap()
    Win = nc.dram_tensor("Win", [F_IN, HID], F16, kind="ExternalInput").ap()
    Whid = nc.dram_tensor("Whid", [HID, HID], F16, kind="ExternalInput").ap()
    Wout = nc.dram_tensor("Wout", [HID, OUT], F16, kind="ExternalInput").ap()
    bin_ = nc.dram_tensor("bin", [HID, 1], F32, kind="ExternalInput").ap()
    bhid = nc.dram_tensor("bhid", [HID, 1], F32, kind="ExternalInput").ap()
    bout = nc.dram_tensor("bout", [OUT, 1], F32, kind="ExternalInput").ap()
    # dinv in src-tile layout: dinvT[p, t] = 1/sqrt(deg[t*128+p])
    dinvTu = nc.dram_tensor("dinvTu", [128, T], F32, kind="ExternalInput").ap()
    dinvTu2 = nc.dram_tensor("dinvTu2", [128, T], F32, kind="ExternalInput").ap()
    # shard dinv replicated across 32 feature partitions
    dinvRu = nc.dram_tensor("dinvRu", [32, DP], F16, kind="ExternalInput").ap()
    dinvRu2 = nc.dram_tensor("dinvRu2", [32, DP], F16, kind="ExternalInput").ap()
    # A blocks laid out so each (dch, g) group is one contiguous [128, TG*w]
    # DMA: Am[dch*NTG + g, p, j*CW + c] = count(src=(g*TG+j)*128+p, dst=dch*CW+c)
    # for the 12 full-width chunks; At[g, p, j*CWT + c] covers dst >= 12*CW.
    Aum = nc.dram_tensor("Aum", [12 * NTG, 128, TG * CW], F8,
                         kind="ExternalInput").ap()
    Aut = nc.dram_tensor("Aut", [NTG, 128, TG * CWT], F8,
                         kind="ExternalInput").ap()
    Au2m = nc.dram_tensor("Au2m", [12 * NTG, 128, TG * CW], F8,
                          kind="ExternalInput").ap()
    Au2t = nc.dram_tensor("Au2t", [NTG, 128, TG * CWT], F8,
                          kind="ExternalInput").ap()
    y1 = nc.dram_tensor("y1", [OUT, DP], F32, kind="ExternalOutput").ap()
    y2 = nc.dram_tensor("y2", [OUT, DP], F32, kind="ExternalOutput").ap()
    # internal DRAM
    ccin = nc.dram_tensor("ccin", [64, SHARD], F16, kind="Internal").ap()
    ccout = nc.dram_tensor("ccout", [N_CORES, 64, SHARD], F16, kind="Internal",
                           addr_space="Shared").ap()

    with tile.TileContext(nc) as tc:
        ctx = contextlib.ExitStack()
        with ctx:
            persist = ctx.enter_context(tc.tile_pool(name="persist", bufs=1))
            stream = ctx.enter_context(tc.tile_pool(name="stream", bufs=2))
            astream = ctx.enter_context(tc.tile_pool(name="astream", bufs=2))
            ps32 = ctx.enter_context(tc.tile_pool(name="ps32", bufs=2, space="PSUM"))
            psaux = ctx.enter_context(tc.tile_pool(name="psaux", bufs=2, space="PSUM"))
            pstr = ctx.enter_context(tc.tile_pool(name="pstr", bufs=2, space="PSUM"))

            hs_nm = persist.tile([128, T * HID], F16, tag="hs_nm")
            hs_nm2 = persist.tile([128, T * HID], F16, tag="hs_nm2")
            x1t = persist.tile([32, DP], F16, tag="x1t")
            x2t = persist.tile([32, DP], F16, tag="x2t")
            m16 = persist.tile([32, DP], F16, tag="m16")     # mian_sh -> later z1 f16
            s16 = persist.tile([32, DP], F16, tag="s16")     # sup_sh -> later z2 f16
            dvT1 = persist.tile([128, T], F32, tag="dvT1")
            dvT2 = persist.tile([128, T], F32, tag="dvT2")
            dvR1 = persist.tile([32, DP], F16, tag="dvR1")
            dvR2 = persist.tile([32, DP], F16, tag="dvR2")
            ones32 = persist.tile([1, 32], F32, tag="ones32")
            ones32c = persist.tile([32, 1], F32, tag="ones32c")
            ident = persist.tile([128, 128], F32, tag="ident")
            ident16 = persist.tile([32, 32], F16, tag="ident16")
            # sum4[32q+i, j] = delta_ij: reduces 4 col-tile partials via PE
            sum4_sb = persist.tile([128, 32], F16, tag="sum4")
            Win_sb = persist.tile([128, 2 * HID], F16, tag="winsb")
            Whid_sb = persist.tile([32, HID], F16, tag="whidsb")
            Wout_sb = persist.tile([32, OUT], F16, tag="woutsb")
            bin_sb = persist.tile([HID, 1], F32, tag="binsb")
            bhid_sb = persist.tile([HID, 1], F32, tag="bhidsb")
            bout_sb = persist.tile([OUT, 1], F32, tag="boutsb")

            nc.vector.memset(ones32[:], 1.0)
            nc.vector.memset(ones32c[:], 1.0)
            make_identity(nc, ident[:])
            nc.vector.tensor_copy(ident16[:], ident[0:32, 0:32])
            for q in range(4):
                nc.sync.dma_start(sum4_sb[32 * q:32 * (q + 1), :], ident16[:])
            nc.sync.dma_start(Win_sb[:, 0:HID], Win[0:128, :])
            nc.sync.dma_start(Win_sb[:, HID:2 * HID], Win[128:256, :])
            nc.sync.dma_start(Whid_sb[:], Whid[:])
            nc.sync.dma_start(Wout_sb[:], Wout[:])
            nc.sync.dma_start(bin_sb[:], bin_[:])
            nc.sync.dma_start(bhid_sb[:], bhid[:])
            nc.sync.dma_start(bout_sb[:], bout[:])
            nc.sync.dma_start(dvT1[:], dinvTu[:])
            nc.sync.dma_start(dvT2[:], dinvTu2[:])
            nc.sync.dma_start(dvR1[:], dinvRu[:])
            nc.sync.dma_start(dvR2[:], dinvRu2[:])

            def hs_store(nch, src_t, dvT, hs_t, dvTb=None, hs_tb=None):
                """Transpose [32, 512] src_t into 4 [128, 32] hs tiles, scaling
                each partition (=src node) by its dinv. Optionally write a
                second scaled copy (shared XW for both L1 branches)."""
                for i in range(4):
                    tci = nch * 4 + i
                    pst = pstr.tile([128, 32], F16, tag="tr")
                    nc.tensor.transpose(out=pst[:],
                                        in_=src_t[:, i * 128:(i + 1) * 128],
                                        identity=ident16[:])
                    nc.vector.tensor_scalar_mul(
                        hs_t[:, tci * 32:(tci + 1) * 32], pst[:],
                        dvT[:, tci:tci + 1])
                    if hs_tb is not None:
                        nc.vector.tensor_scalar_mul(
                            hs_tb[:, tci * 32:(tci + 1) * 32], pst[:],
                            dvTb[:, tci:tci + 1])

            def build_hs_L1():
                """hs_nm/hs_nm2 <- dinv_{u,u2} * (X @ Win), shared XW."""
                for b in range(NCH // 2):
                    xts = []
                    for k in range(2):
                        xt = stream.tile([128, 2 * CW], F16,
                                         tag="xt%d" % k)
                        nc.sync.dma_start(
                            xt[:], xT[k * 128:(k + 1) * 128,
                                      b * 2 * CW:(b + 1) * 2 * CW])
                        xts.append(xt)
                    for i in range(2):
                        nch = 2 * b + i
                        csl = slice(i * CW, (i + 1) * CW)
                        psh = psaux.tile([32, CW], F32, tag="aux")
                        for k in range(2):
                            nc.tensor.matmul(
                                out=psh[:],
                                lhsT=Win_sb[:, k * HID:(k + 1) * HID],
                                rhs=xts[k][:, csl],
                                start=(k == 0), stop=(k == 1))
                        scr = stream.tile([32, CW], F16, tag="scr")
                        nc.scalar.copy(scr[:], psh[:])
                        hs_store(nch, scr, dvT1, hs_nm, dvT2, hs_nm2)

            def build_hs_L23_blocks(branch_row, with_W, dvT, hs_t):
                """Per-block emitters for an hs build from gathered tables
                (ccout[(core), branch_row:+32, .]). Returning closures lets
                callers interleave emission into a conv's instruction
                stream (engines execute in program order, so a build
                emitted as its own phase blocks the next conv's PE ops)."""
                def make_emit(b):
                    def emit():
                        tabt = stream.tile([32, NB * CW], F16, tag="tabt")
                        col0 = b * NB * CW
                        ncols = NB * CW
                        if col0 < N:
                            for (c, so, do, ln) in _segments(
                                    col0, min(ncols, N - col0)):
                                nc.sync.dma_start(
                                    tabt[:, do:do + ln],
                                    ccout[c, branch_row:branch_row + 32,
                                          so:so + ln])
                        if col0 + ncols > N:
                            z0 = max(N - col0, 0)
                            nc.vector.memset(tabt[:, z0:], 0.0)
                        for i in range(NB):
                            nch = b * NB + i
                            sl = slice(i * CW, (i + 1) * CW)
                            if with_W:
                                psh = psaux.tile([32, CW], F32, tag="aux")
                                nc.tensor.matmul(out=psh[:], lhsT=Whid_sb[:],
                                                 rhs=tabt[:, sl], start=True,
                                                 stop=True)
                                scr = stream.tile([32, CW], F16, tag="scr")
                                nc.scalar.copy(scr[:], psh[:])
                                hs_store(nch, scr, dvT, hs_t)
                            else:
                                hs_store(nch, tabt[:, sl], dvT, hs_t)
                    return emit
                return [make_emit(b) for b in range(NCH // NB)]

            def build_hs_L23(branch_row, with_W, dvT, hs_t):
                for em in build_hs_L23_blocks(branch_row, with_W, dvT, hs_t):
                    em()

            def evict(ps, w, dvR, sl, b_sb, out_t):
                """out_t[:, sl] = (sum of 4 col-tile partials) * dinv (+ b).
                Cross-partition reduce must go through the PE (engines are
                lane-aligned): sum4^T @ cp."""
                cp = stream.tile([128, CW], F16, tag="cp16")
                nc.scalar.copy(cp[:, 0:w], ps[:, 0:w])
                ps2 = ps32.tile([32, CW], F32, tag="psh")
                nc.tensor.matmul(out=ps2[:, 0:w], lhsT=sum4_sb[:],
                                 rhs=cp[:, 0:w], start=True, stop=True)
                t3 = stream.tile([32, CW], F32, tag="wide3")
                nc.scalar.copy(t3[:, 0:w], ps2[:, 0:w])
                t4 = stream.tile([32, CW], F32, tag="wide1")
                nc.vector.tensor_tensor(out=t4[:, 0:w], in0=t3[:, 0:w],
                                        in1=dvR[:, sl],
                                        op=mybir.AluOpType.mult)
                if b_sb is not None:
                    nc.vector.tensor_scalar_add(out_t[:, sl], t4[:, 0:w],
                                                b_sb[:])
                else:
                    nc.vector.tensor_copy(out_t[:, sl], t4[:, 0:w])

            def a_block(A_aps, dch, g):
                Am, At = A_aps
                w = CWS[dch]
                sbA = astream.tile([128, TG * w], F8,
                                   tag="sbA" if w == CW else "sbAt")
                src = Am[dch * NTG + g] if w == CW else At[g]
                nc.sync.dma_start(sbA[:], src)
                return sbA, w

            def conv(A_aps, hs_t, dvR, b_sb, out_t, post=None):
                """out_t[32, DP] = dinvsh * (hs^T @ A) (+ b), 4-way col-tiled.
                post(dch) lets dependent per-chunk work (boundary math)
                interleave into the DMA-bound stream."""
                if variant in ("noconv", "minimal"):
                    nc.vector.memset(out_t[:], 0.0)
                    return
                for dch in range(DCH):
                    w = CWS[dch]
                    psa = ps32.tile([128, CW], F32, tag="ps32")
                    for g in range(NTG):
                        sbA, w = a_block(A_aps, dch, g)
                        for j in range(TG):
                            t = g * TG + j
                            q = (t % 4) * 32
                            nc.tensor.matmul(
                                out=psa[q:q + 32, 0:w],
                                lhsT=hs_t[:, t * HID:(t + 1) * HID],
                                rhs=sbA[:, j * w:(j + 1) * w],
                                start=(t < 4), stop=(t >= T - 4),
                                tile_position=(0, q))
                    evict(psa, w, dvR, slice(CS[dch], CS[dch] + w), b_sb,
                          out_t)
                    if post is not None:
                        post(dch)

            def conv2(A_aps, dvR, out_a, out_b, post=None, pre=None):
                """Fused pair: stream A once, matmul against hs_nm and
                hs_nm2, write both scaled outputs. pre(dch, g) lets hs
                builds emit just-in-time ahead of each matmul group."""
                if variant in ("noconv", "minimal"):
                    nc.vector.memset(out_a[:], 0.0)
                    nc.vector.memset(out_b[:], 0.0)
                    return
                for dch in range(DCH):
                    w = CWS[dch]
                    psa = ps32.tile([128, CW], F32, tag="ps32")
                    psb = psaux.tile([128, CW], F32, tag="aux")
                    for g in range(NTG):
                        if pre is not None:
                            pre(dch, g)
                        sbA, w = a_block(A_aps, dch, g)
                        for j in range(TG):
                            t = g * TG + j
                            q = (t % 4) * 32
                            nc.tensor.matmul(
                                out=psa[q:q + 32, 0:w],
                                lhsT=hs_nm[:, t * HID:(t + 1) * HID],
                                rhs=sbA[:, j * w:(j + 1) * w],
                                start=(t < 4), stop=(t >= T - 4),
                                tile_position=(0, q))
                            nc.tensor.matmul(
                                out=psb[q:q + 32, 0:w],
                                lhsT=hs_nm2[:, t * HID:(t + 1) * HID],
                                rhs=sbA[:, j * w:(j + 1) * w],
                                start=(t < 4), stop=(t >= T - 4),
                                tile_position=(0, q))
                    sl = slice(CS[dch], CS[dch] + w)
                    evict(psa, w, dvR, sl, None, out_a)
                    evict(psb, w, dvR, sl, None, out_b)
                    if post is not None:
                        post(dch)

            def boundary_chunk(dch):
                """x1t/x2t chunk (fp16) -> m16 = mian, s16 = sup (fp16)."""
                if True:
                    w = CWS[dch]
                    sl = slice(CS[dch], CS[dch] + w)
                    ra = stream.tile([1, CW], F32, tag="browa")
                    rb = stream.tile([1, CW], F32, tag="browb")
                    # ra <- n1^2, rb <- n2^2; then ra <- 1/max(sqrt(ra*rb),eps)
                    for k, (a, b, dst) in enumerate(((x1t, x1t, ra),
                                                     (x2t, x2t, rb))):
                        tm = stream.tile([32, CW], F32, tag="wide1")
                        nc.vector.tensor_tensor(out=tm[:, 0:w], in0=a[:, sl],
                                                in1=b[:, sl],
                                                op=mybir.AluOpType.mult)
                        pss = psaux.tile([1, CW], F32, tag="aux")
                        nc.tensor.matmul(out=pss[:, 0:w], lhsT=ones32c[:],
                                         rhs=tm[:, 0:w], start=True, stop=True)
                        nc.scalar.copy(dst[:, 0:w], pss[:, 0:w])
                    nc.vector.tensor_tensor(out=ra[:, 0:w], in0=ra[:, 0:w],
                                            in1=rb[:, 0:w],
                                            op=mybir.AluOpType.mult)
                    nc.scalar.sqrt(ra[:, 0:w], ra[:, 0:w])
                    nc.vector.tensor_scalar_max(ra[:, 0:w], ra[:, 0:w], EPS)
                    nc.vector.reciprocal(ra[:, 0:w], ra[:, 0:w])
                    # rb <- dot, then rb <- sim = dot * ra
                    tm = stream.tile([32, CW], F32, tag="wide1")
                    nc.vector.tensor_tensor(out=tm[:, 0:w], in0=x1t[:, sl],
                                            in1=x2t[:, sl],
                                            op=mybir.AluOpType.mult)
                    pss = psaux.tile([1, CW], F32, tag="aux")
                    nc.tensor.matmul(out=pss[:, 0:w], lhsT=ones32c[:],
                                     rhs=tm[:, 0:w], start=True, stop=True)
                    nc.scalar.copy(rb[:, 0:w], pss[:, 0:w])
                    nc.vector.tensor_tensor(out=rb[:, 0:w], in0=rb[:, 0:w],
                                            in1=ra[:, 0:w],
                                            op=mybir.AluOpType.mult)
                    psr = psaux.tile([32, CW], F32, tag="aux")
                    nc.tensor.matmul(out=psr[:, 0:w], lhsT=ones32[:],
                                     rhs=rb[:, 0:w], start=True, stop=True)
                    srp = stream.tile([32, CW], F32, tag="wide2")
                    nc.scalar.copy(srp[:, 0:w], psr[:, 0:w])
                    tm1 = stream.tile([32, CW], F32, tag="wide3")
                    nc.vector.tensor_tensor(out=tm1[:, 0:w], in0=x2t[:, sl],
                                            in1=srp[:, 0:w],
                                            op=mybir.AluOpType.mult)
                    nc.vector.tensor_tensor(out=m16[:, sl], in0=x1t[:, sl],
                                            in1=tm1[:, 0:w],
                                            op=mybir.AluOpType.add)
                    tm2 = stream.tile([32, CW], F32, tag="wide3")
                    nc.vector.tensor_tensor(out=tm2[:, 0:w], in0=x1t[:, sl],
                                            in1=srp[:, 0:w],
                                            op=mybir.AluOpType.mult)
                    nc.vector.tensor_tensor(out=s16[:, sl], in0=x2t[:, sl],
                                            in1=tm2[:, 0:w],
                                            op=mybir.AluOpType.add)

            def allgather():
                nc.sync.dma_start(ccin[0:32, :], m16[:, 0:SHARD])
                nc.sync.dma_start(ccin[32:64, :], s16[:, 0:SHARD])
                if variant in ("noag", "minimal"):
                    for c in range(N_CORES):
                        nc.sync.dma_start(ccout[c], ccin[:])
                else:
                    nc.gpsimd.collective_compute(
                        "AllGather", mybir.AluOpType.bypass,
                        replica_groups=[list(range(N_CORES))],
                        ins=[ccin[:]], outs=[ccout[:]])

            def final_chunk(zf, y_ap, dch):
                w = CWS[dch]
                sl = slice(CS[dch], CS[dch] + w)
                psy = ps32.tile([128, CW], F32, tag="ps32")
                nc.tensor.matmul(out=psy[:, 0:w], lhsT=Wout_sb[:],
                                 rhs=zf[:, sl], start=True, stop=True)
                yst = stream.tile([128, CW], F32, tag="yst")
                nc.scalar.copy(yst[:, 0:w], psy[:, 0:w])
                nc.vector.tensor_scalar_add(yst[:, 0:w], yst[:, 0:w],
                                            bout_sb[:])
                nc.sync.dma_start(y_ap[:, sl], yst[:, 0:w])

            def final_both(dch):
                final_chunk(m16, y1, dch)
                final_chunk(s16, y2, dch)

            # ================= schedule =================
            if variant == "minimal":
                nc.vector.memset(m16[:], 0.0)
                nc.vector.memset(s16[:], 0.0)
                for dch in range(DCH):
                    final_both(dch)
            else:
                Au = (Aum, Aut)
                Au2 = (Au2m, Au2t)
                build_hs_L1()
                conv(Au, hs_nm, dvR1, bin_sb, x1t)
                # boundary math rides along conv_b's DMA-bound stream
                conv(Au2, hs_nm2, dvR2, bin_sb, x2t, post=boundary_chunk)
                allgather()

                build_hs_L23(0, True, dvT1, hs_nm)
                # u2 build rides along conv-L2-a's DMA-bound stream: ~4
                # blocks per dst chunk keeps hs_nm2 well ahead of conv-L2-b
                ems = build_hs_L23_blocks(32, True, dvT2, hs_nm2)
                nper = -(-len(ems) // DCH)

                def post_l2a(dch):
                    for em in ems[dch * nper:(dch + 1) * nper]:
                        em()
                conv(Au, hs_nm, dvR1, bhid_sb, x1t, post=post_l2a)
                conv(Au2, hs_nm2, dvR2, bhid_sb, x2t, post=boundary_chunk)
                allgather()

                # L3 builds emit just-in-time inside conv2's first dst
                # chunk: group g consumes hs tiles g*TG..g*TG+TG-1, i.e.
                # 7 four-tile blocks per group, of BOTH branches.
                ems1 = build_hs_L23_blocks(0, False, dvT1, hs_nm)
                ems2 = build_hs_L23_blocks(32, False, dvT1, hs_nm2)

                def pre3(dch, g):
                    if dch == 0:
                        for em in ems1[7 * g:7 * (g + 1)]:
                            em()
                        for em in ems2[7 * g:7 * (g + 1)]:
                            em()
                conv2(Au, dvR1, m16, s16, post=final_both, pre=pre3)

    nc.compile()
    _cache[key] = nc
    return nc


def _prep_inputs(x, ei_u, ei_u2, W_in, W_hid, W_out, b_in, b_hid, b_out):
    f8lut = np.arange(64, dtype=np.float32).astype(ml_dtypes.float8_e4m3)

    xT16 = np.zeros((F_IN, NP), np.float16)
    xT16[:, :N] = np.asarray(x, np.float32).T

    def deg_of(ei):
        d = np.bincount(np.asarray(ei[1], np.int64), minlength=N)
        return (d + 1.0).astype(np.float32)

    dinv_u = 1.0 / np.sqrt(deg_of(ei_u))
    dinv_u2 = 1.0 / np.sqrt(deg_of(ei_u2))

    def dinvT_layout(dinv):
        out = np.ones(NP, np.float32)
        out[:N] = dinv
        return np.ascontiguousarray(out.reshape(T, 128).T)

    def dinvR_layout(dinv, c):
        out = np.ones(DP, np.float32)
        out[:SHARD] = dinv[c * SHARD:(c + 1) * SHARD]
        return np.broadcast_to(out.astype(np.float16), (32, DP)).copy()

    def build_A(ei, c):
        lo, hi = c * SHARD, (c + 1) * SHARD
        src = np.asarray(ei[0], np.int64)
        dst = np.asarray(ei[1], np.int64)
        m = (dst >= lo) & (dst < hi)
        s, d = src[m], dst[m] - lo
        g = np.arange(lo, hi, dtype=np.int64)
        s = np.concatenate([s, g])          # self-loops on the diagonal
        d = np.concatenate([d, g - lo])

        def flat_main(s_, d_):
            # layout [dch, g, p, j, c]: see Aum comment in build_program
            return (((d_ >> 9) * NTG + s_ // (TG * 128)) * (128 * TG * CW)
                    + (s_ % 128) * (TG * CW)
                    + ((s_ // 128) % TG) * CW + (d_ & 511))

        def flat_tail(s_, d_):
            return ((s_ // (TG * 128)) * (128 * TG * CWT)
                    + (s_ % 128) * (TG * CWT)
                    + ((s_ // 128) % TG) * CWT + (d_ - 12 * CW))

        mt = d >= 12 * CW
        bufm = np.zeros(12 * NTG * 128 * TG * CW, np.uint8)
        np.add.at(bufm, flat_main(s[~mt], d[~mt]), 1)
        buft = np.zeros(NTG * 128 * TG * CWT, np.uint8)
        np.add.at(buft, flat_tail(s[mt], d[mt]), 1)
        return (f8lut[np.minimum(bufm, 63)].reshape(12 * NTG, 128, TG * CW),
                f8lut[np.minimum(buft, 63)].reshape(NTG, 128, TG * CWT))

    common = {
        "xT": xT16,
        "Win": np.asarray(W_in, np.float32).astype(np.float16),
        "Whid": np.asarray(W_hid, np.float32).astype(np.float16),
        "Wout": np.asarray(W_out, np.float32).astype(np.float16),
        "bin": np.asarray(b_in, np.float32).reshape(HID, 1),
        "bhid": np.asarray(b_hid, np.float32).reshape(HID, 1),
        "bout": np.asarray(b_out, np.float32).reshape(OUT, 1),
        "dinvTu": dinvT_layout(dinv_u), "dinvTu2": dinvT_layout(dinv_u2),
    }
    per_core = []
    for c in range(N_CORES):
        im = dict(common)
        im["dinvRu"] = dinvR_layout(dinv_u, c)
        im["dinvRu2"] = dinvR_layout(dinv_u2, c)
        im["Aum"], im["Aut"] = build_A(ei_u, c)
        im["Au2m"], im["Au2t"] = build_A(ei_u2, c)
        per_core.append(im)
    return per_core


last_results = None
_last_in_maps = None


def bench(iters=10, nc=None, in_maps=None):
    """Measure marginal per-execution device time by chaining executions
    through donated output buffers (subtracts the client-tunnel latency,
    which is ~85 ms per unpipelined round trip)."""
    import time
    import jax
    from jax.sharding import Mesh, PartitionSpec, NamedSharding
    from jax.experimental.shard_map import shard_map
    from concourse import bass2jax, mybir as _mb

    nc = nc or _cache[("nc", "full")]
    in_maps = in_maps or _last_in_maps
    bass2jax.install_neuronx_cc_hook()
    partition_name = (nc.partition_id_tensor.name
                      if nc.partition_id_tensor else None)
    in_names, out_names, out_avals, zero_outs = [], [], [], []
    for alloc in nc.m.functions[0].allocations:
        if not isinstance(alloc, _mb.MemoryLocationSet):
            continue
        name = alloc.memorylocations[0].name
        if alloc.kind == "ExternalInput":
            if name != partition_name:
                in_names.append(name)
        elif alloc.kind == "ExternalOutput":
            shape = tuple(alloc.tensor_shape)
            dtype = _mb.dt.np(alloc.dtype)
            out_names.append(name)
            out_avals.append(jax.core.ShapedArray(shape, dtype))
            zero_outs.append(np.zeros(shape, dtype))
    n_params = len(in_names)
    n_outs = len(out_avals)
    all_in_names = list(in_names) + list(out_names)
    if partition_name is not None:
        all_in_names.append(partition_name)

    def _body(*args):
        operands = list(args)
        if partition_name is not None:
            operands.append(bass2jax.partition_id_tensor())
        outs = bass2jax._bass_exec_p.bind(
            *operands,
            out_avals=tuple(out_avals),
            in_names=tuple(all_in_names),
            out_names=tuple(out_names),
            lowering_input_output_aliases=(),
            sim_require_finite=False,
            sim_require_nnan=False,
            nc=nc,
        )
        return tuple(outs)

    devices = jax.devices()[:N_CORES]
    mesh = Mesh(np.asarray(devices), ("core",))
    spec = PartitionSpec("core")
    in_specs = (spec,) * (n_params + n_outs)
    out_specs = (spec,) * n_outs
    jf = jax.jit(shard_map(_body, mesh=mesh, in_specs=in_specs,
                           out_specs=out_specs, check_rep=False),
                 donate_argnums=tuple(range(n_params, n_params + n_outs)),
                 keep_unused=True)
    sharding = NamedSharding(mesh, spec)
    concat_in = [
        jax.device_put(
            np.concatenate([np.asarray(in_maps[c][n]) for c in range(N_CORES)],
                           axis=0), sharding)
        for n in in_names
    ]
    for a in concat_in:
        a.block_until_ready()
    zo = [jax.device_put(
        np.zeros((N_CORES * z.shape[0], *z.shape[1:]), z.dtype), sharding)
        for z in zero_outs]
    for a in zo:
        a.block_until_ready()
    # warm-up (includes NEFF load)
    cur = jf(*concat_in, *zo)
    for o in cur:
        o.block_until_ready()
    # single chained call (latency-dominated)
    t1 = None
    for _ in range(2):
        t0 = time.perf_counter()
        cur = jf(*concat_in, *cur)
        for o in cur:
            o.block_until_ready()
        dt = time.perf_counter() - t0
        t1 = dt if t1 is None else min(t1, dt)
    # N chained calls (pipelined through donated outputs); best of 2 chains
    tN = None
    for _ in range(2):
        t0 = time.perf_counter()
        for _ in range(iters):
            cur = jf(*concat_in, *cur)
        for o in cur:
            o.block_until_ready()
        dt = time.perf_counter() - t0
        tN = dt if tN is None else min(tN, dt)
    marginal = max(tN - t1, 0.0) / max(iters - 1, 1)
    return {"single_s": t1, "chainN_s": tN, "iters": iters,
            "marginal_s": marginal}


def kernel(x, edge_index_u, edge_index_u2, W_in, b_in, W_hid, b_hid,
           W_out, b_out):
    global last_results, _last_in_maps
    nc = build_program()
    in_maps = _prep_inputs(x, edge_index_u, edge_index_u2,
                           W_in, W_hid, W_out, b_in, b_hid, b_out)
    _last_in_maps = in_maps
    res = bass_utils.run_bass_kernel_spmd(nc, in_maps,
                                          core_ids=list(range(N_CORES)))
    last_results = res
    out = np.zeros((N, 2 * OUT), np.float32)
    for c in range(N_CORES):
        lo, hi = c * SHARD, (c + 1) * SHARD
        out[lo:hi, 0:OUT] = res.results[c]["y1"][:, :SHARD].T
        out[lo:hi, OUT:2 * OUT] = res.results[c]["y2"][:, :SHARD].T
    return out
